# revision 12
# baseline (speedup 1.0000x reference)
"""Trainium2 Bass kernel for nn_CkConv1D (continuous-kernel causal conv).

Math: the reference builds a T x T Toeplitz kernel K[o,c,i,j] =
sum_h w2[h]*sin(A_h*(j-i) + off[o,c,h]) + b2  (A_h = w1[h,0]/T), masks it
causally (j<=i) and contracts with x [T, C].  Since K depends only on
(j - i), everything is phrased in LOCAL window coordinates (ii = i mod 128,
jj = j mod 128): with theta[p,(o,ii)] = off0_p + o*w12_p - A_p*ii,

  y[i,o] = sum_p w2_p * [cos(theta)*(pwS + histS)_p + sin(theta)*(pwC + histC)_p]
           + b2 * (pwx + pcx)[ii]

where pwS/pwC are causal window prefix sums of sin(A jj)*x / cos(A jj)*x
(one upper-triangular matmul each), and the history term comes from
per-block partial sums P[(b,c),(t,h)] = xblk^T @ [sin|cos](A jj) rotated by
block phases 128*A*(b-m) and summed over blocks b<m (tiny masked matmul).

Sharded over 8 NeuronCores: core m computes output rows [128m, 128m+128).
SPMD: identical program, per-core behavior comes only from input data
(x window slice, block-phase grids, block mask).  Host prep is limited to
layout/replication and affine iota*weight phase grids (pre-wrapped into
[-pi, pi) because the ACT Sin LUT is only accurate there); all sines,
x contractions and T^2-scale work happen on device.

Partition layout: p = c*32 + h (C_in=4 channels x H=32 hidden = 128).
"""

import sys
from pathlib import Path

import numpy as np

for _p in ("/opt/trn_rl_repo",):
    if _p not in sys.path and Path(_p).exists():
        sys.path.insert(0, _p)

import concourse.bass as bass
import concourse.bacc as bacc
import concourse.tile as tile
from concourse import mybir
from concourse.bass_utils import run_bass_kernel_spmd

F32 = mybir.dt.float32
F32R = mybir.dt.float32r
BF16 = mybir.dt.bfloat16
F16 = mybir.dt.float16
PI = float(np.pi)
PI2 = float(np.pi / 2)
T, C, O, H, P, M = 1024, 4, 2, 32, 128, 8

# D2 (2-byte tensor) column offsets.  bf16 columns hold bf16 data; "grid"
# columns hold raw fp16 bits (bitcast to F16 on device before ACT).
D_UT = 0          # [128, 128] bf16 upper-tri (jj <= ii)
D_XWIN = 128      # [128, 4]   bf16 own x window
D_MASK4 = 132     # [32, 128]  bf16 mask4[(b,c),(c',h)] = (c==c')&(b<m)
D_B2 = 260        # [4, 1]     bf16 b2 (rows 0:4)
D_N = 261

# G32 (fp32 phase-grid tensor) columns.  All ACT inputs are fp32: the Sin
# LUT needs two serialized table loads for fp16 inputs but only one for
# fp32, and the tables are on the critical path head.
G_QTSG = 0        # [128, 512] wrap(theta) | wrap(theta + pi/2)
G_ARGL = 512      # [128, 64]  A_h*jj | A_h*jj + pi/2
G_ARGB = 576      # [32, 64]   wrap(128*A*(b-m)) | wrap(... + pi/2) (rows 0:32)
G_N = 640

# DW (float32r tensor) columns
W_W2 = 0          # [128, 1] w2 tiled over c
W_E4M = 1         # [32, 4]  E4M[(b,c), c'] = (c==c') & (b<m)  (rows 0:32)
W_XBLK = 5        # [128, 32] x blocked [jj, (b,c)] (plain fp32 bits)
W_N = 37

ACT_F32R = True   # Scalar ACT writes float32r directly (else cast on DVE)

_nc_cache = {}


def _build_nc():
    nc = bacc.Bacc()
    d2 = nc.dram_tensor("d2", [P, D_N], BF16, kind="ExternalInput")
    dw = nc.dram_tensor("dw", [P, W_N], F32R, kind="ExternalInput")
    g32 = nc.dram_tensor("g32", [P, G_N], F32, kind="ExternalInput")
    y = nc.dram_tensor("y", [1, O, P], F32, kind="ExternalOutput")

    Sin = mybir.ActivationFunctionType.Sin
    QT_DT = F32R if ACT_F32R else F32

    with tile.TileContext(nc) as tc:
        with (
            tc.tile_pool(name="sb", bufs=1) as sb,
            tc.tile_pool(name="ps", bufs=1, space="PSUM") as ps,
        ):
            d2_sb = sb.tile([P, D_N], BF16)
            dw_sb = sb.tile([P, W_N], F32R)
            g32_sb = sb.tile([P, G_N], F32)
            nc.sync.dma_start(out=g32_sb[:], in_=g32[:])
            nc.scalar.dma_start(out=d2_sb[:], in_=d2[:])
            nc.scalar.dma_start(out=dw_sb[:], in_=dw[:])

            def grid(off, n, rows=P):
                return g32_sb[0:rows, off:off + n]

            # dummy sin with no upstream deps: forces the ACT Sin table
            # load to happen at t=0 instead of serializing behind the DMAs
            pi2c = sb.tile([P, 1], F32)
            nc.vector.memset(pi2c[:], PI2)
            warm = sb.tile([P, 1], F32)
            nc.scalar.activation(warm[:], pi2c[:], Sin)

            # ---- trig tables (one fused ACT per grid pair) ----
            TLp = sb.tile([P, 2 * H + 1], F32)    # [jj, sin|cos|ones]
            nc.scalar.activation(TLp[:, 0:2 * H], grid(G_ARGL, 2 * H), Sin)
            nc.vector.memset(TLp[:, 2 * H:2 * H + 1], 1.0)
            phSC = sb.tile([H, 2 * H], F32)       # block phases [(b,c), h]
            nc.scalar.activation(phSC[:], grid(G_ARGB, 2 * H, rows=H), Sin)
            phS, phC = phSC[:, 0:H], phSC[:, H:2 * H]
            QT = sb.tile([P, 2, O, P], QT_DT)     # query side [p, s|c, o, ii]
            nc.scalar.activation(
                QT[:].rearrange("p t o i -> p (t o i)"),
                grid(G_QTSG, 2 * O * P), Sin)
            QTs, QTc = QT[:, 0], QT[:, 1]

            # ---- window products R[jj, (c,h)] = trig[jj,h] * xwin[jj,c] ----
            R_s = sb.tile([P, C, H], BF16)
            R_c = sb.tile([P, C, H], BF16)
            tl_s = TLp[:, 0:H].unsqueeze(1).broadcast_to([P, C, H])
            tl_c = TLp[:, H:2 * H].unsqueeze(1).broadcast_to([P, C, H])
            xw_b = d2_sb[:, D_XWIN:D_XWIN + C].unsqueeze(2).broadcast_to([P, C, H])
            nc.vector.tensor_mul(R_s[:], tl_s, xw_b)
            nc.vector.tensor_mul(R_c[:], tl_c, xw_b)

            # ---- PE: history partials, then window prefix sums ----
            p_ps = ps.tile([H, 2 * H + 1], F32)   # P[(b,c), (sin|cos,h)|ones]
            nc.tensor.matmul(p_ps[:], dw_sb[:, W_XBLK:W_XBLK + H].bitcast(F32),
                             TLp[:], start=True, stop=True)
            ut = d2_sb[:, D_UT:D_UT + P]
            pwS = ps.tile([P, P], F32)
            pwC = ps.tile([P, P], F32)
            pwx = ps.tile([C, P], F32)
            nc.tensor.matmul(pwS[:], R_s[:], ut, start=True, stop=True)
            nc.tensor.matmul(pwC[:], R_c[:], ut, start=True, stop=True)
            nc.tensor.matmul(pwx[:], d2_sb[:, D_XWIN:D_XWIN + C], ut,
                             start=True, stop=True)

            # ---- history: rotate partials by block phases, mask+sum b<m ----
            # The masked sum over b lands directly in [(c,h)] partitions:
            # replicate Q over c' with the (c==c')&(b<m) mask on DVE, then
            # contract the 32 (b,c)-partitions against a ones column on PE
            # (out partitions = stationary free dim).  No transpose DMA.
            Ps, Pc = p_ps[:, 0:H], p_ps[:, H:2 * H]
            t_a = sb.tile([H, H], F32)
            t_b = sb.tile([H, H], F32)
            Qs_t = sb.tile([H, H], F32)
            Qc_t = sb.tile([H, H], F32)
            nc.vector.tensor_mul(t_a[:], phC, Ps)
            nc.vector.tensor_mul(t_b[:], phS, Pc)
            nc.vector.tensor_add(Qs_t[:], t_a[:], t_b[:])
            nc.vector.tensor_mul(t_a[:], phC, Pc)
            nc.vector.tensor_mul(t_b[:], phS, Ps)
            nc.vector.tensor_sub(Qc_t[:], t_a[:], t_b[:])
            m4 = d2_sb[0:H, D_MASK4:D_MASK4 + P].rearrange(
                "p (c h) -> p c h", c=C)
            Qs4 = sb.tile([H, C, H], BF16)
            Qc4 = sb.tile([H, C, H], BF16)
            nc.vector.tensor_mul(Qs4[:], Qs_t[:].unsqueeze(1).broadcast_to([H, C, H]), m4)
            nc.vector.tensor_mul(Qc4[:], Qc_t[:].unsqueeze(1).broadcast_to([H, C, H]), m4)
            ones32 = sb.tile([H, 1], BF16)
            nc.vector.memset(ones32[:], 1.0)
            pones = sb.tile([H, 1], F32)
            nc.vector.tensor_copy(pones[:], p_ps[:, 2 * H:2 * H + 1])
            hist2 = ps.tile([P, 2], F32)
            qflat = lambda ap: ap.rearrange("p c h -> p (c h)")
            nc.tensor.matmul(hist2[:, 0:1], qflat(Qs4[:]), ones32[:],
                             start=True, stop=True)
            nc.tensor.matmul(hist2[:, 1:2], qflat(Qc4[:]), ones32[:],
                             start=True, stop=True)
            pcx4 = ps.tile([C, 1], F32)
            nc.tensor.matmul(pcx4[:], dw_sb[0:H, W_E4M:W_E4M + C].bitcast(F32),
                             pones[:], start=True, stop=True)

            # ---- combine on DVE (no col dependency: hist goes via PE) ----
            G1 = sb.tile([P, O, P], F32R)
            G2 = sb.tile([P, O, P], F32R)
            pwS_b = pwS[:].unsqueeze(1).broadcast_to([P, O, P])
            pwC_b = pwC[:].unsqueeze(1).broadcast_to([P, O, P])
            nc.vector.tensor_mul(G1[:], pwS_b, QTc)
            nc.vector.tensor_mul(G2[:], pwC_b, QTs)
            QTs_r, QTc_r = QTs, QTc
            # b2 term: t4x2[c, (o,ii)] = pwx + pcx, replicated over o
            t4x2 = sb.tile([C, O, P], BF16)
            pwx_b = pwx[:].unsqueeze(1).broadcast_to([C, O, P])
            nc.vector.tensor_scalar_add(t4x2[:], pwx_b, pcx4[:])
            # w2-scaled hist columns feed the final contraction directly
            wcol_s = sb.tile([P, 1], F32R)
            wcol_c = sb.tile([P, 1], F32R)
            nc.vector.tensor_mul(wcol_s[:], dw_sb[:, W_W2:W_W2 + 1], hist2[:, 0:1])
            nc.vector.tensor_mul(wcol_c[:], dw_sb[:, W_W2:W_W2 + 1], hist2[:, 1:2])

            # ---- final contraction over p (and c for the b2 term) ----
            yterm = ps.tile([1, O * P], F32)
            w2col = dw_sb[:, W_W2:W_W2 + 1]
            flat = lambda ap: ap.rearrange("p o i -> p (o i)")
            nc.tensor.matmul(yterm[:], w2col, flat(G1[:]), start=True, stop=False)
            nc.tensor.matmul(yterm[:], w2col, flat(G2[:]), start=False, stop=False)
            nc.tensor.matmul(yterm[:], wcol_s[:], flat(QTc_r), start=False, stop=False)
            nc.tensor.matmul(yterm[:], wcol_c[:], flat(QTs_r), start=False, stop=False)
            nc.tensor.matmul(yterm[:], d2_sb[0:C, D_B2:D_B2 + 1],
                             flat(t4x2[:]), start=False, stop=True)
            ysb = sb.tile([1, O * P], F32)
            nc.vector.tensor_copy(ysb[:], yterm[:])
            nc.sync.dma_start(out=y[:].rearrange("p o i -> p (o i)"), in_=ysb[:])
    nc.finalize()
    return nc


def _wrap(v):
    return (v + np.pi) % (2 * np.pi) - np.pi


def _host_inputs(x, w1, b1, w2, b2):
    """Per-core input maps.  Host does layout/replication/masking and
    affine iota*weight phase grids (pre-wrapped for the LUT range)."""
    bf16 = mybir.dt.np(BF16)
    x = np.ascontiguousarray(x, np.float32)
    w1 = np.asarray(w1, np.float64)
    b1 = np.asarray(b1, np.float64)
    w2 = np.asarray(w2, np.float32)
    b2 = np.asarray(b2, np.float32)

    A = w1[:, 0] / T                                    # [H]
    jj = np.arange(P, dtype=np.float64)
    argL = np.outer(jj, A)                              # [128, 32]
    cidx = np.arange(C, dtype=np.float64)
    off0 = cidx[:, None] * w1[None, :, 1] + b1[None, :] # [C, H]
    oidx = np.arange(O, dtype=np.float64)
    theta = (off0[:, :, None, None]
             + oidx[None, None, :, None] * w1[None, :, 2, None, None]
             - A[None, :, None, None] * jj[None, None, None, :])  # [C,H,O,P]
    qtsg = _wrap(theta).reshape(P, O * P)
    qtcg = _wrap(theta + np.pi / 2).reshape(P, O * P)

    ut = np.triu(np.ones((P, P), np.float32))
    xblk = x.reshape(M, P, C).transpose(1, 0, 2).reshape(P, M * C)

    d2_base = np.zeros((P, D_N), dtype=bf16)
    d2_base[:, D_UT:D_UT + P] = ut.astype(bf16)
    d2_base[0:C, D_B2] = np.full(C, b2[0]).astype(bf16)

    g32_base = np.zeros((P, G_N), np.float32)
    g32_base[:, G_QTSG:G_QTSG + O * P] = qtsg
    g32_base[:, G_QTSG + O * P:G_QTSG + 2 * O * P] = qtcg
    g32_base[:, G_ARGL:G_ARGL + H] = argL
    g32_base[:, G_ARGL + H:G_ARGL + 2 * H] = argL + np.pi / 2

    dw_base = np.zeros((P, W_N), np.float32)
    dw_base[:, W_W2] = np.tile(w2[0], C)
    dw_base[:, W_XBLK:W_XBLK + H] = xblk

    bvals = np.repeat(np.arange(M, dtype=np.float64), C)  # [(b,c)] -> b
    cvals = np.tile(np.arange(C), M)                      # [(b,c)] -> c
    in_maps = []
    for m in range(M):
        d2m = d2_base.copy()
        d2m[:, D_XWIN:D_XWIN + C] = x[P * m:P * (m + 1)].astype(bf16)
        g32m = g32_base.copy()
        argB = np.outer(P * (bvals - m), A)               # [32, 32]
        g32m[0:H, G_ARGB:G_ARGB + H] = _wrap(argB)
        g32m[0:H, G_ARGB + H:G_ARGB + 2 * H] = _wrap(argB + np.pi / 2)
        dwm = dw_base.copy()
        e4m = (cvals[:, None] == np.arange(C)[None, :]) & (bvals[:, None] < m)
        dwm[0:H, W_E4M:W_E4M + C] = e4m.astype(np.float32)
        mask4 = np.repeat(e4m, H, axis=1)                 # [32, 128]
        d2m[0:H, D_MASK4:D_MASK4 + P] = mask4.astype(bf16)
        in_maps.append({"d2": d2m, "dw": dwm, "g32": g32m})
    return in_maps


def kernel(x, t, w1, b1, w2, b2, out_channels):
    if "nc" not in _nc_cache:
        _nc_cache["nc"] = _build_nc()
    nc = _nc_cache["nc"]
    in_maps = _host_inputs(x, w1, b1, w2, b2)
    res = run_bass_kernel_spmd(nc, in_maps, core_ids=list(range(M)))
    y = np.empty((T, O), np.float32)
    for m in range(M):
        ym = np.asarray(res.results[m]["y"]).reshape(O, P)
        y[P * m:P * (m + 1), :] = ym.T
    return y
